# revision 5
# baseline (speedup 1.0000x reference)
"""Trainium2 Bass kernel for nn_AnalyticalTC_33414845563115.

Computes, for each (b, c):
  out1[b,c,i,j] = sum_k T1[b,c,i*3+k] * T2[b,k*9+j] / sqrt(3)   (i,j in 0..9)
  out2[b,c,i,j] = sum_k T1[b,c,i*9+k] * T2[b,k*3+j] / 3         (i,j in 0..3)
  out3[b,c]     = sum_d T1[b,c,d]     * T2[b,d]     / sqrt(27)

Sharding: pure data parallel over B across 8 NeuronCores.
Device layout: b on partitions (tiles of 128), per-partition-scalar
fused MACs on the vector engine.
"""

import sys

sys.path.insert(0, "/opt/trn_rl_repo")

from contextlib import ExitStack

import numpy as np

import concourse.bass as bass
import concourse.tile as tile
from concourse import bacc, mybir
from concourse.bass import ts
from concourse.bass_utils import run_bass_kernel_spmd

B, C, D = 20000, 64, 27
N_CORES = 8
B_CORE = B // N_CORES  # 2500
TILE_P = 128
N_TILES = (B_CORE + TILE_P - 1) // TILE_P  # 20
B_PAD = N_TILES * TILE_P  # 2560

F32 = mybir.dt.float32


def build_kernel():
    nc = bacc.Bacc()
    T1 = nc.declare_dram_parameter("t1", [B_PAD, C * D], F32, isOutput=False)
    # T2h: per-b row of 81 = [T2/sqrt(3) (27) | T2/3 (27) | T2/sqrt(27) (27)],
    # stored partition-major: [128, N_TILES * 81]
    T2 = nc.declare_dram_parameter("t2", [TILE_P, N_TILES * 81], F32, isOutput=False)
    O1 = nc.declare_dram_parameter("o1", [B_PAD, C * 81], F32, isOutput=True)
    O2 = nc.declare_dram_parameter("o2", [B_PAD, C * 9], F32, isOutput=True)
    O3 = nc.declare_dram_parameter("o3", [TILE_P, N_TILES * C], F32, isOutput=True)

    with TileContextCompat(nc) as tc, ExitStack() as ctx:
        in_pool = ctx.enter_context(tc.tile_pool(name="in", bufs=3))
        o1_pool = ctx.enter_context(tc.tile_pool(name="o1", bufs=3))
        o2_pool = ctx.enter_context(tc.tile_pool(name="o2", bufs=3))
        const_pool = ctx.enter_context(tc.tile_pool(name="const", bufs=1))

        t2_tile = const_pool.tile([TILE_P, N_TILES * 81], F32)
        nc.sync.dma_start(t2_tile[:], T2[:, :])
        o3_tile = const_pool.tile([TILE_P, N_TILES * C], F32)

        for t in range(N_TILES):
            t1_tile = in_pool.tile([TILE_P, C * D], F32)
            nc.sync.dma_start(t1_tile[:], T1[ts(t, TILE_P), :])

            o1_tile = o1_pool.tile([TILE_P, C * 81], F32)
            o2_tile = o2_pool.tile([TILE_P, C * 9], F32)

            # Gate ops: TensorScalarPtr instructions only have room for one
            # sync-wait, but the first op touching fresh tiles needs to wait
            # on several DMA semaphores. Absorb those waits into TensorTensor
            # gate ops; subsequent same-engine ops then follow in program
            # order without semaphores.
            nc.vector.tensor_mul(o1_tile[:, 0:1], t1_tile[:, 0:1], t2_tile[:, 0:1])
            nc.vector.tensor_mul(o2_tile[:, 0:1], t1_tile[:, 0:1], t2_tile[:, 0:1])

            # views
            t1_m1 = t1_tile[:].rearrange("p (c i k) -> p c i k", c=C, i=9, k=3)
            t1_m2 = t1_tile[:].rearrange("p (c i k) -> p c i k", c=C, i=3, k=9)
            o1_v = o1_tile[:].rearrange("p (c i j) -> p c i j", c=C, i=9, j=9)
            o2_v = o2_tile[:].rearrange("p (c i j) -> p c i j", c=C, i=3, j=3)

            def s(idx):
                return t2_tile[:, t * 81 + idx : t * 81 + idx + 1]

            # out1: for each j: acc over k
            for j in range(9):
                out = o1_v[:, :, :, j : j + 1]
                for k in range(3):
                    in0 = t1_m1[:, :, :, k : k + 1]
                    sc = s(k * 9 + j)
                    if k == 0:
                        nc.vector.tensor_scalar(
                            out, in0, sc, None, mybir.AluOpType.mult
                        )
                    else:
                        nc.vector.scalar_tensor_tensor(
                            out, in0, sc, out,
                            mybir.AluOpType.mult, mybir.AluOpType.add,
                        )

            # out2: for each j: acc over k (9)
            for j in range(3):
                out = o2_v[:, :, :, j : j + 1]
                for k in range(9):
                    in0 = t1_m2[:, :, :, k : k + 1]
                    sc = s(27 + k * 3 + j)
                    if k == 0:
                        nc.vector.tensor_scalar(
                            out, in0, sc, None, mybir.AluOpType.mult
                        )
                    else:
                        nc.vector.scalar_tensor_tensor(
                            out, in0, sc, out,
                            mybir.AluOpType.mult, mybir.AluOpType.add,
                        )

            # out3: acc over d (27)
            out = o3_tile[:, ts(t, C)]
            t1_m3 = t1_tile[:].rearrange("p (c d) -> p c d", c=C, d=D)
            for d in range(D):
                in0 = t1_m3[:, :, d : d + 1]
                sc = s(54 + d)
                out2d = out.rearrange("p (c one) -> p c one", c=C, one=1)
                if d == 0:
                    nc.vector.tensor_scalar(out2d, in0, sc, None, mybir.AluOpType.mult)
                else:
                    nc.vector.scalar_tensor_tensor(
                        out2d, in0, sc, out2d,
                        mybir.AluOpType.mult, mybir.AluOpType.add,
                    )

            nc.sync.dma_start(O1[ts(t, TILE_P), :], o1_tile[:])
            nc.sync.dma_start(O2[ts(t, TILE_P), :], o2_tile[:])

        nc.sync.dma_start(O3[:, :], o3_tile[:])

    nc.finalize()
    return nc


def TileContextCompat(nc):
    return tile.TileContext(nc)


def _prep_core_inputs(t1c: np.ndarray, t2c: np.ndarray):
    """t1c: (B_CORE, C, D) f32; t2c: (B_CORE, 1, D) f32 -> in_map dict."""
    t1p = np.zeros((B_PAD, C * D), dtype=np.float32)
    t1p[:B_CORE] = t1c.reshape(B_CORE, C * D)

    t2flat = t2c.reshape(B_CORE, D).astype(np.float32)
    t2h = np.zeros((B_PAD, 81), dtype=np.float32)
    t2h[:B_CORE, 0:27] = t2flat * np.float32(1.0 / np.sqrt(3.0))
    t2h[:B_CORE, 27:54] = t2flat * np.float32(1.0 / 3.0)
    t2h[:B_CORE, 54:81] = t2flat * np.float32(1.0 / np.sqrt(27.0))
    # partition-major: [128, N_TILES*81]
    t2pm = np.ascontiguousarray(
        t2h.reshape(N_TILES, TILE_P, 81).transpose(1, 0, 2).reshape(TILE_P, N_TILES * 81)
    )
    return {"t1": t1p, "t2": t2pm}


def kernel(T1: np.ndarray, T2: np.ndarray):
    T1 = np.asarray(T1, dtype=np.float32)
    T2 = np.asarray(T2, dtype=np.float32)
    assert T1.shape == (B, C, D) and T2.shape == (B, 1, D)

    nc = build_kernel()
    in_maps = []
    for c in range(N_CORES):
        sl = slice(c * B_CORE, (c + 1) * B_CORE)
        in_maps.append(_prep_core_inputs(T1[sl], T2[sl]))

    res = run_bass_kernel_spmd(nc, in_maps, core_ids=list(range(N_CORES)))
    outs = res.results

    o1 = np.empty((B, C, 81), dtype=np.float32)
    o2 = np.empty((B, C, 9), dtype=np.float32)
    o3 = np.empty((B, C), dtype=np.float32)
    for c in range(N_CORES):
        sl = slice(c * B_CORE, (c + 1) * B_CORE)
        r = outs[c]
        o1[sl] = r["o1"][:B_CORE].reshape(B_CORE, C, 81)
        o2[sl] = r["o2"][:B_CORE].reshape(B_CORE, C, 9)
        o3c = (
            r["o3"].reshape(TILE_P, N_TILES, C).transpose(1, 0, 2).reshape(B_PAD, C)
        )
        o3[sl] = o3c[:B_CORE]

    out1 = o1.reshape(B, C, 3, 3, 3, 3)
    out2 = o2.reshape(B, C, 3, 3)
    out3 = o3
    return (out1, out2, out3)


# revision 43
# speedup vs baseline: 195.5843x; 195.5843x over previous
"""Trainium2 Bass kernel for nn_AnalyticalTC_33414845563115.

Per (b, c), with T1 (B,C,27), T2 (B,1,27):
  out1[b,c,i,j] = sum_k3 T1[b,c,i*3+k] * T2[b,k*9+j] / sqrt(3)   (9x9)
  out2[b,c,i,j] = sum_k9 T1[b,c,i*9+k] * T2[b,k*3+j] / 3         (3x3)
  out3[b,c]     = sum_d  T1[b,c,d]     * T2[b,d]     / sqrt(27)

Sharding: pure data parallel over B across 8 NeuronCores (2500 b each,
padded to 2520 = 60 groups x 42).

Device strategy per group of G=42 batch elements:
  - T1 staged once as t1pe[(g,k3) 126 partitions, (c,i9) 576 free]
    (host pre-transposed; one layout serves all three contractions).
  - out1: 6 matmuls [126,126]x[126,288] on PE with host-built
    block-diagonal T2 weights (j in chunks of 3, c in halves).
  - out2: 3 PSUM-accumulating matmuls with strided rhs views.
  - out3: 9 scalar_tensor_tensor MACs (per-partition scalars, DVE/GPSIMD
    alternating by group) + one block-diag-of-ones matmul to sum over the
    k partitions.
  - PSUM evictions (with no extra scaling; norms folded into weights)
    split between DVE and ACT; outputs DMAed in PE-friendly layout and
    rearranged on the host.
"""

import sys

sys.path.insert(0, "/opt/trn_rl_repo")

from contextlib import ExitStack

import numpy as np

import concourse.bass as bass
import concourse.tile as tile
from concourse import bacc, mybir
from concourse.bass_utils import run_bass_kernel_spmd

B, C, D = 20000, 64, 27
N_CORES = 8
B_CORE = B // N_CORES  # 2500
G = 42  # batch elements per group
NP = 3 * G  # 126 partitions in use
N_GROUPS = (B_CORE + G - 1) // G  # 60
B_PAD = N_GROUPS * G  # 2520

F32 = mybir.dt.float32

# PE compute dtype: "float32" (exact, 4 cyc/row), "bfloat16" (1 cyc/row),
# "float32r" (1 cyc/row for N>=256, reduced precision).
PE_DTYPE = "float32"

# Build block-diagonal weights on device (mask x broadcast-T2) instead of
# DMAing them dense from HBM (saves ~23 MB DMA per core).
BUILD_BD = True


def _pe_dt():
    return getattr(mybir.dt, PE_DTYPE)


def _pe_np():
    import ml_dtypes

    return {"float32": np.float32, "float32r": np.float32,
            "bfloat16": ml_dtypes.bfloat16}[PE_DTYPE]


def build_kernel():
    pdt = _pe_dt()
    nc = bacc.Bacc()
    T1P = nc.declare_dram_parameter("t1pe", [N_GROUPS, NP, C * 9], pdt, isOutput=False)
    if BUILD_BD:
        T2M1 = nc.declare_dram_parameter(
            "t2m1", [NP, N_GROUPS * 9], F32, isOutput=False
        )
        T2M2 = nc.declare_dram_parameter(
            "t2m2", [NP, N_GROUPS * 9], F32, isOutput=False
        )
        MASK = nc.declare_dram_parameter("mask", [NP, NP], F32, isOutput=False)
    else:
        BD1 = nc.declare_dram_parameter(
            "bd1", [N_GROUPS, NP, 3 * NP], pdt, isOutput=False
        )
        BD2 = nc.declare_dram_parameter(
            "bd2", [N_GROUPS, NP, 3 * NP], pdt, isOutput=False
        )
    T2X = nc.declare_dram_parameter("t2x", [NP, N_GROUPS * 9], F32, isOutput=False)
    ONES = nc.declare_dram_parameter("ones3", [NP, G], F32, isOutput=False)
    O1 = nc.declare_dram_parameter("o1", [N_GROUPS, NP, 6 * 288], F32, isOutput=True)
    O2 = nc.declare_dram_parameter("o2", [N_GROUPS, NP, 192], F32, isOutput=True)
    O3 = nc.declare_dram_parameter("o3", [G, N_GROUPS * C], F32, isOutput=True)

    mult, add = mybir.AluOpType.mult, mybir.AluOpType.add

    with tile.TileContext(nc) as tc, ExitStack() as ctx:
        t1_pool = ctx.enter_context(tc.tile_pool(name="t1", bufs=4))
        bd_pool = ctx.enter_context(tc.tile_pool(name="bd", bufs=4))
        o1_pool = ctx.enter_context(tc.tile_pool(name="o1", bufs=3))
        o2_pool = ctx.enter_context(tc.tile_pool(name="o2", bufs=3))
        tmp_pool = ctx.enter_context(tc.tile_pool(name="tmp", bufs=3))
        const_pool = ctx.enter_context(tc.tile_pool(name="const", bufs=1))
        psum1_pool = ctx.enter_context(tc.tile_pool(name="ps1", bufs=7, space="PSUM"))
        psum3_pool = ctx.enter_context(tc.tile_pool(name="ps3", bufs=1, space="PSUM"))

        o3_tile = const_pool.tile([G, N_GROUPS * C], F32)
        mask_tile = const_pool.tile([NP, NP], F32)
        t2m1_tile = const_pool.tile([NP, N_GROUPS * 9], F32)
        t2m2_tile = const_pool.tile([NP, N_GROUPS * 9], F32)
        t2x_tile = const_pool.tile([NP, N_GROUPS * 9], F32)
        ones_tile = const_pool.tile([NP, G], F32)
        if BUILD_BD:
            # [126, jc(bcast), g', j']
            mask_v = (
                mask_tile[:]
                .rearrange("p (g j) -> p g j", g=G, j=3)
                .unsqueeze(1)
                .broadcast_to([NP, 3, G, 3])
            )

        for g in range(N_GROUPS):
            t1_tile = t1_pool.tile([NP, C * 9], pdt)
            nc.sync.dma_start(t1_tile[:], T1P[g, :, :])
            if g == 0:
                # consts after the first T1 tile so PE starts ASAP
                if BUILD_BD:
                    nc.sync.dma_start(mask_tile[:], MASK[:, :])
                    nc.sync.dma_start(t2m1_tile[:], T2M1[:, :])
                    nc.sync.dma_start(t2m2_tile[:], T2M2[:, :])
                nc.sync.dma_start(t2x_tile[:], T2X[:, :])
                nc.sync.dma_start(ones_tile[:], ONES[:, :])
            bd1_tile = bd_pool.tile([NP, 3 * NP], pdt, tag="bd1")
            bd2_tile = bd_pool.tile([NP, 3 * NP], pdt, tag="bd2")
            if BUILD_BD:
                # bd1 = mask * broadcast(T2 row block), on DVE
                t2b = (
                    t2m1_tile[:, g * 9 : (g + 1) * 9]
                    .rearrange("p (jc j) -> p jc j", jc=3, j=3)
                    .unsqueeze(2)
                    .broadcast_to([NP, 3, G, 3])
                )
                bd1_v = bd1_tile[:].rearrange(
                    "p (jc gp j) -> p jc gp j", jc=3, gp=G, j=3
                )
                nc.vector.tensor_mul(bd1_v, mask_v, t2b)
                # bd2 on GPSIMD
                t2b2 = (
                    t2m2_tile[:, g * 9 : (g + 1) * 9]
                    .rearrange("p (kc j) -> p kc j", kc=3, j=3)
                    .unsqueeze(2)
                    .broadcast_to([NP, 3, G, 3])
                )
                bd2_v = bd2_tile[:].rearrange(
                    "p (kc gp j) -> p kc gp j", kc=3, gp=G, j=3
                )
                nc.gpsimd.tensor_mul(bd2_v, mask_v, t2b2)
            else:
                nc.sync.dma_start(bd1_tile[:], BD1[g, :, :])
                nc.sync.dma_start(bd2_tile[:], BD2[g, :, :])

            if g % 2 == 0:
                o1_big = o1_pool.tile([NP, 2 * 6 * 288], F32)
            o1_tile = o1_big[:, (g % 2) * 6 * 288 : (g % 2 + 1) * 6 * 288]
            if g % 4 == 0:
                o2_big = o2_pool.tile([NP, 4 * 192], F32)
            o2_tile = o2_big[:, (g % 4) * 192 : (g % 4 + 1) * 192]
            t1_v = t1_tile[:].rearrange("p (c i) -> p c i", c=C, i=9)

            # ---- out1: 6 matmuls (3 j-chunks x 2 c-halves) ----
            for jc in range(3):
                for ch in range(2):
                    ps = psum1_pool.tile([NP, 288], F32, tag="ps1")
                    rhs = t1_tile[:, ch * 288 : (ch + 1) * 288]
                    nc.tensor.matmul(
                        ps[:],
                        bd1_tile[:, jc * NP : (jc + 1) * NP],
                        rhs,
                        start=True,
                        stop=True,
                    )
                    # all psum evictions on ACT; DVE owns bd1-build + STT
                    dst = o1_tile[:, (jc * 2 + ch) * 288 : (jc * 2 + ch + 1) * 288]
                    nc.scalar.copy(dst, ps[:])

            # ---- out2: 3 accumulating matmuls over a (=i9%3) ----
            ps2_t = psum1_pool.tile([NP, 288], F32, tag="ps1")
            ps2 = ps2_t[:, 0:192]
            for kc in range(3):
                # rhs: free (c:64 stride 9)(i':3 stride 3) offset kc
                rhs = t1_v[:, :, kc::3]
                nc.tensor.matmul(
                    ps2[:],
                    bd2_tile[:, kc * NP : (kc + 1) * NP],
                    rhs,
                    start=(kc == 0),
                    stop=(kc == 2),
                )
            nc.scalar.copy(o2_tile[:], ps2[:])

            # ---- out3: 9 STT MACs + ones-matmul over k-partitions ----
            tmp3 = tmp_pool.tile([NP, C], F32)
            for i in range(9):
                in0 = t1_v[:, :, i : i + 1]
                sc = t2x_tile[:, g * 9 + i : g * 9 + i + 1]
                t3v = tmp3[:].rearrange("p (c one) -> p c one", c=C, one=1)
                if i == 0:
                    nc.vector.tensor_scalar(t3v, in0, sc, None, mult)
                else:
                    nc.vector.scalar_tensor_tensor(t3v, in0, sc, t3v, mult, add)
            ps3 = psum3_pool.tile([G, C], F32)
            nc.tensor.matmul(ps3[:], ones_tile[:], tmp3[:], start=True, stop=True)
            nc.scalar.copy(o3_tile[:, g * C : (g + 1) * C], ps3[:])

            if g % 2 == 1:
                nc.scalar.dma_start(
                    O1[g - 1 : g + 1, :, :].transpose([1, 0, 2]), o1_big[:].rearrange("p (a f) -> p a f", a=2, f=6*288)
                )
            if g % 4 == 3:
                nc.scalar.dma_start(
                    O2[g - 3 : g + 1, :, :].transpose([1, 0, 2]), o2_big[:].rearrange("p (a f) -> p a f", a=4, f=192)
                )
            # flush o3 in 4 chunks so the final DMA isn't all in the tail
            if g in (14, 29, 44, N_GROUPS - 1):
                lo = (g // 15) * 15
                nc.sync.dma_start(
                    O3[:, lo * C : (g + 1) * C], o3_tile[:, lo * C : (g + 1) * C]
                )

    nc.finalize()
    return nc


def _prep_core_inputs(t1c: np.ndarray, t2c: np.ndarray):
    """t1c: (B_CORE, C, D) f32; t2c: (B_CORE, 1, D) f32 -> in_map dict."""
    pnp = _pe_np()
    t1p = np.zeros((B_PAD, C, D), dtype=np.float32)
    t1p[:B_CORE] = t1c
    t2p = np.zeros((B_PAD, D), dtype=np.float32)
    t2p[:B_CORE] = t2c.reshape(B_CORE, D)

    n1 = np.float32(1.0 / np.sqrt(3.0))
    n2 = np.float32(1.0 / 3.0)
    n3 = np.float32(1.0 / np.sqrt(27.0))

    # t1pe[G, g*3+k, c*9+i] = T1[42G+g, c, i*3+k]
    t1pe = np.ascontiguousarray(
        t1p.reshape(N_GROUPS, G, C, 9, 3).transpose(0, 1, 4, 2, 3)
    ).reshape(N_GROUPS, NP, C * 9).astype(pnp)

    gi = np.arange(G)
    t2v = t2p.reshape(N_GROUPS, G, D)
    if BUILD_BD:
        # t2m1[p=(g,k), G*9 + jj] = T2[b, k*9+jj] * n1
        t2m1 = np.ascontiguousarray(
            (t2p.reshape(N_GROUPS, G, 3, 9) * n1).transpose(1, 2, 0, 3)
        ).reshape(NP, N_GROUPS * 9).astype(np.float32)
        # t2m2[p=(g,km), G*9 + (kc*3+j')] = T2[b, 9kc+3km+j'] * n2
        t2m2 = np.ascontiguousarray(
            (t2p.reshape(N_GROUPS, G, 3, 3, 3) * n2).transpose(1, 3, 0, 2, 4)
        ).reshape(NP, N_GROUPS * 9).astype(np.float32)
        mask = np.zeros((NP, NP), dtype=np.float32)
        for k in range(3):
            for jp in range(3):
                mask[gi * 3 + k, gi * 3 + jp] = 1.0
        extra = {"t2m1": t2m1, "t2m2": t2m2, "mask": mask}
    else:
        # bd1[G, g*3+k, jc*126 + g*3+j'] = T2[b, k*9+3jc+j'] * n1
        bd1 = np.zeros((N_GROUPS, NP, 3 * NP), dtype=np.float32)
        for k in range(3):
            for jc in range(3):
                for jp in range(3):
                    bd1[:, gi * 3 + k, jc * NP + gi * 3 + jp] = (
                        t2v[:, gi, k * 9 + 3 * jc + jp] * n1
                    )
        # bd2[G, g*3+km, kc*126 + g*3+j'] = T2[b, (3kc+km)*3+j'] * n2
        bd2 = np.zeros((N_GROUPS, NP, 3 * NP), dtype=np.float32)
        for km in range(3):
            for kc in range(3):
                for jp in range(3):
                    bd2[:, gi * 3 + km, kc * NP + gi * 3 + jp] = (
                        t2v[:, gi, (3 * kc + km) * 3 + jp] * n2
                    )
        extra = {"bd1": bd1.astype(pnp), "bd2": bd2.astype(pnp)}

    # t2x[g*3+k, G*9+i] = T2[42G+g, i*3+k] * n3
    t2x = np.ascontiguousarray(
        (t2p.reshape(N_GROUPS, G, 9, 3) * n3).transpose(1, 3, 0, 2)
    ).reshape(NP, N_GROUPS * 9).astype(np.float32)

    ones3 = np.zeros((NP, G), dtype=np.float32)
    ones3[gi * 3 + 0, gi] = 1.0
    ones3[gi * 3 + 1, gi] = 1.0
    ones3[gi * 3 + 2, gi] = 1.0

    return {"t1pe": t1pe, "t2x": t2x, "ones3": ones3, **extra}


def _unpack_core_outputs(r):
    """Device layouts -> (out1 (B_PAD,C,9,9), out2 (B_PAD,C,3,3), out3 (B_PAD,C))."""
    # o1[G, g*3+j', (jc,ch,cc32,i9)] = out1[b, ch*32+cc, i, 3jc+j']
    o1 = r["o1"].reshape(N_GROUPS, G, 3, 3, 2, 32, 9)  # [G,g,j',jc,ch,cc,i]
    # -> [G, g, ch, cc, i, jc, j'] so that c=(ch,cc), j=(jc,j')
    out1 = o1.transpose(0, 1, 4, 5, 6, 3, 2).reshape(B_PAD, C, 9, 9)
    # o2[G, g*3+j', (c,i')] = out2[b, c, i', j']
    o2 = r["o2"].reshape(N_GROUPS, G, 3, C, 3)  # [G,g,j',c,i']
    out2 = o2.transpose(0, 1, 3, 4, 2).reshape(B_PAD, C, 3, 3)
    # o3[g', G*64+c] = out3[42G+g', c]
    o3 = r["o3"].reshape(G, N_GROUPS, C)
    out3 = o3.transpose(1, 0, 2).reshape(B_PAD, C)
    return out1, out2, out3


def kernel(T1: np.ndarray, T2: np.ndarray):
    T1 = np.asarray(T1, dtype=np.float32)
    T2 = np.asarray(T2, dtype=np.float32)
    assert T1.shape == (B, C, D) and T2.shape == (B, 1, D)

    nc = build_kernel()
    in_maps = []
    for c in range(N_CORES):
        sl = slice(c * B_CORE, (c + 1) * B_CORE)
        in_maps.append(_prep_core_inputs(T1[sl], T2[sl]))

    res = run_bass_kernel_spmd(nc, in_maps, core_ids=list(range(N_CORES)))

    out1 = np.empty((B, C, 9, 9), dtype=np.float32)
    out2 = np.empty((B, C, 3, 3), dtype=np.float32)
    out3 = np.empty((B, C), dtype=np.float32)
    for c in range(N_CORES):
        sl = slice(c * B_CORE, (c + 1) * B_CORE)
        a1, a2, a3 = _unpack_core_outputs(res.results[c])
        out1[sl] = a1[:B_CORE]
        out2[sl] = a2[:B_CORE]
        out3[sl] = a3[:B_CORE]

    return (
        out1.reshape(B, C, 3, 3, 3, 3),
        out2,
        out3,
    )
